# revision 1
# baseline (speedup 1.0000x reference)
"""CocktailGNN (3-layer GraphSAGE, mean aggregation) on 8 trn2 NeuronCores.

Strategy: shard by *destination-node range* (edge lists sorted by dst).
Each core owns N/8 = 6250 dst nodes. Per layer:
  - every core holds the full node-feature table (f32 for layer-1 input,
    bf16 for layers 2/3) in DRAM, rebuilt each layer via AllGather;
  - gather of neighbor rows via indirect DMA (128 rows x C chunks per
    128-dst-node block);
  - segment-sum via one-hot matmul on the TensorEngine: the one-hot is
    built on the VectorEngine with a broadcast is_equal against an iota;
  - inv-degree (mean) scaling fused into the PSUM->SBUF copy;
  - SAGE linears (agg @ Wl + h @ Wr + bl, relu) as dense matmuls with
    fused bias+relu on the ScalarEngine, all in transposed [F, nodes]
    layout so bias is per-partition.
"""

import os
import sys

sys.path.insert(0, "/opt/trn_rl_repo")
os.environ.setdefault("MYCRO_LOCAL_CACHE", "1")

import numpy as np

import concourse.bass as bass
import concourse.bacc as bacc
import concourse.mybir as mybir
import concourse.tile as tile
from concourse import bass_utils
from concourse.bass import IndirectOffsetOnAxis
from concourse.masks import make_identity

P = 128
F32 = mybir.dt.float32
BF16 = mybir.dt.bfloat16
I32 = mybir.dt.int32
NP_BF16 = mybir.dt.np(BF16)

USE_SHARED_TABLE = os.environ.get("GNN_SHARED_TABLE", "1") == "1"
# table dtype for layers 2/3: f16 (default), bf16, or f32
TABLE_DT = os.environ.get("GNN_TABLE_DT", "f16")
if os.environ.get("GNN_TABLE_BF16") == "0":  # legacy flag
    TABLE_DT = "f32"


class Cfg:
    def __init__(self, N=50000, E=800000, in_dim=2, emb=128, hid=256, r=8):
        assert N % r == 0
        self.N, self.E, self.IN_DIM, self.EMB, self.HID, self.R = N, E, in_dim, emb, hid, r
        self.NPC = N // r                      # nodes per core
        self.B = (self.NPC + P - 1) // P       # 128-node blocks per core
        self.NPB = self.B * P                  # padded nodes per core
        self.TROWS = r * self.NPB              # table rows


HALF = 32768  # int16 index ceiling for dma_gather


def host_prep(cfg: Cfg, x, edge_index, W_in, b_in, layers):
    """Build the per-core input maps. layers = [(Wl, bl, Wr)] * 3.

    Edges are bucketed by (core, 128-dst-node block) and, within a block,
    split into region A (src table row < HALF) and region B (>= HALF) so
    that dma_gather's int16 indices can address the whole table via two
    calls (second reads from table[HALF:]). Each region is padded to a
    fixed chunk count (C_A/C_B x 128 slots) so the program is SPMD-uniform;
    pad slots gather row 0 of their half and carry d_rel = -1 so the
    one-hot kills their contribution.
    """
    src = np.asarray(edge_index[0], dtype=np.int64)
    dst = np.asarray(edge_index[1], dtype=np.int64)
    N, E, R, B, NPC, NPB = cfg.N, cfg.E, cfg.R, cfg.B, cfg.NPC, cfg.NPB

    deg = np.bincount(dst, minlength=N).astype(np.float64)
    inv = np.where(deg > 0, 1.0 / np.maximum(deg, 1), 0.0).astype(np.float32)

    core = dst // NPC
    local = dst % NPC
    blk = local // P
    drel = (local % P).astype(np.float32)
    group = core * B + blk

    srow_all = ((src // NPC) * NPB + (src % NPC)).astype(np.int64)
    region = (srow_all >= HALF).astype(np.int64)
    grp2 = group * 2 + region

    cnt2 = np.bincount(grp2, minlength=R * B * 2)
    cntA = cnt2[0::2]
    cntB = cnt2[1::2]
    def round4(c):
        return ((c + 3) // 4) * 4

    C_A = round4(max(1, int(np.ceil(cntA.max() / P))))
    C_B = int(np.ceil(cntB.max() / P))
    if C_B > 0:
        C_B = round4(C_B)
    S_A, S_B = C_A * P, C_B * P

    order = np.argsort(grp2, kind="stable")
    offs = np.zeros(R * B * 2 + 1, np.int64)
    np.cumsum(cnt2, out=offs[1:])
    g2s = grp2[order]
    pos = np.arange(E) - offs[g2s]
    gs = group[order]
    regs = region[order].astype(bool)
    srow_s = srow_all[order]
    drel_s = drel[order]

    mA, mB = ~regs, regs
    flatA = gs[mA] * S_A + pos[mA]
    flatB = gs[mB] * S_B + pos[mB]

    idxA = np.zeros(R * B * S_A, np.int16)
    idxA[flatA] = srow_s[mA].astype(np.int16)
    idxB = np.zeros(R * B * S_B, np.int16)
    idxB[flatB] = (srow_s[mB] - HALF).astype(np.int16)

    drelA = np.full(R * B * S_A, -1.0, np.float32)
    drelA[flatA] = drel_s[mA]
    drelB = np.full(R * B * S_B, -1.0, np.float32)
    drelB[flatB] = drel_s[mB]

    # d_rel device layout: [128, B*CT], column b*CT + c, partition = slot % 128
    CT = C_A + C_B
    dA = drelA.reshape(R, B, C_A, P)
    dB = drelB.reshape(R, B, C_B, P)
    drel_dev = np.concatenate([dA, dB], axis=2).transpose(0, 3, 1, 2).reshape(R, P, B * CT)

    # idx device layout: index i of a region lives at [i % 16, i // 16],
    # replicated across the 8 groups of 16 partitions.
    def idx_dev(arr, S):
        a = arr.reshape(R, B, S // 16, 16).transpose(0, 3, 1, 2).reshape(R, 16, B * (S // 16))
        return np.ascontiguousarray(np.tile(a, (1, 8, 1)))

    idxA_dev = idx_dev(idxA, S_A)
    idxB_dev = idx_dev(idxB, S_B) if C_B > 0 else None

    invrow = np.zeros((R, 1, NPB), np.float32)
    invrow[:, 0, :NPC] = inv.reshape(R, NPC)

    xT = np.zeros((R, cfg.IN_DIM, NPB), np.float32)
    xT[:, :, :NPC] = np.asarray(x, np.float32).reshape(R, NPC, cfg.IN_DIM).transpose(0, 2, 1)

    in_maps = []
    for r in range(R):
        m = {
            "xT": xT[r],
            "invrow": invrow[r],
            "idxA": idxA_dev[r],
            **({"idxB": idxB_dev[r]} if C_B > 0 else {}),
            "drel": np.ascontiguousarray(drel_dev[r]),
            "w_in": np.asarray(W_in, np.float32),
            "b_in": np.asarray(b_in, np.float32),
        }
        for li, (Wl, bl, Wr) in enumerate(layers, start=1):
            m[f"wl{li}"] = np.asarray(Wl, np.float32)
            m[f"bl{li}"] = np.asarray(bl, np.float32)
            m[f"wr{li}"] = np.asarray(Wr, np.float32)
        in_maps.append(m)
    return in_maps, (C_A, C_B)


def emit(tc: tile.TileContext, outs, ins, cfg: Cfg, C, repeat=1):
    C_A, C_B = C
    CT = C_A + C_B
    nc = tc.nc
    B, NPB, TROWS = cfg.B, cfg.NPB, cfg.TROWS
    IN_DIM, EMB, HID, R = cfg.IN_DIM, cfg.EMB, cfg.HID, cfg.R
    F16 = mybir.dt.float16
    tbl_dt = {"f16": F16, "bf16": BF16, "f32": F32}[TABLE_DT]
    lowp = TABLE_DT != "f32"
    groups = [list(range(R))]
    addr_space = "Shared" if (USE_SHARED_TABLE and R > 4) else "Local"

    # node chunks for the dense-linear phase
    t_chunks = []
    t0 = 0
    while t0 < NPB:
        w = min(512, NPB - t0)
        t_chunks.append((t0, w))
        t0 += w

    from contextlib import ExitStack
    ctx = ExitStack()
    const = ctx.enter_context(tc.tile_pool(name="const", bufs=1))
    work = ctx.enter_context(tc.tile_pool(name="work", bufs=2))
    psA = ctx.enter_context(tc.tile_pool(name="psA", bufs=1, space="PSUM"))
    dram = ctx.enter_context(tc.tile_pool(name="dram", bufs=1, space="DRAM"))

    f = F32

    # ---------------- persistent SBUF ----------------
    hT = [const.tile([P, NPB], f, name=f"hT{k}") for k in range(HID // P)]
    aggT = [const.tile([P, NPB], f, name=f"aggT{k}") for k in range(HID // P)]
    invrep = const.tile([P, NPB], f, name="invrep")
    iota_i = const.tile([P, P], I32, name="iota_i")
    iota_f = const.tile([P, P], f, name="iota_f")
    iota_b = const.tile([P, P], tbl_dt if lowp else BF16, name="iota_b")
    ident = const.tile([P, P], f, name="ident")
    I16 = mybir.dt.int16
    idxA_sb = const.tile([P, B * (C_A * P // 16)], I16, name="idxA_sb")
    idxB_sb = const.tile([P, B * (C_B * P // 16)], I16, name="idxB_sb") if C_B > 0 else None
    drel_f = const.tile([P, B * CT], f, name="drel_f")
    drel_b = const.tile([P, B * CT], tbl_dt if lowp else BF16, name="drel_b")
    ones_sb = const.tile([1, P], f, name="ones_sb")
    w_in_sb = const.tile([IN_DIM, EMB], f, name="w_in_sb")
    b_in_sb = const.tile([EMB, 1], f, name="b_in_sb")

    wl_sb, wr_sb, bl_sb, f_ins = {}, {}, {}, {}
    for li in range(1, 4):
        fin = EMB if li == 1 else HID
        f_ins[li] = fin
        wl_sb[li] = [const.tile([P, HID], f, name=f"wl{li}_{k}") for k in range(fin // P)]
        wr_sb[li] = [const.tile([P, HID], f, name=f"wr{li}_{k}") for k in range(fin // P)]
        bl_sb[li] = const.tile([P, HID // P], f, name=f"bl{li}_sb")

    # ---------------- DRAM tables & bounce buffers ----------------
    table0 = dram.tile([TROWS, EMB], f, addr_space=addr_space, name="table0")
    table1 = dram.tile([TROWS, HID], tbl_dt, addr_space=addr_space, name="table1")
    table2 = dram.tile([TROWS, HID], tbl_dt, addr_space=addr_space, name="table2")
    rows0 = dram.tile([NPB, EMB], f, name="rows0")
    rows1 = dram.tile([NPB, HID], tbl_dt, name="rows1")
    rows2 = dram.tile([NPB, HID], tbl_dt, name="rows2")

    # ---------------- load constants ----------------
    nc.sync.dma_start(idxA_sb[:], ins["idxA"][:])
    if C_B > 0:
        nc.sync.dma_start(idxB_sb[:], ins["idxB"][:])
    nc.sync.dma_start(drel_f[:], ins["drel"][:])
    nc.vector.tensor_copy(drel_b[:], drel_f[:])
    nc.sync.dma_start(w_in_sb[:], ins["w_in"][:])
    nc.sync.dma_start(b_in_sb[:], ins["b_in"][:, None])
    for li in range(1, 4):
        fin = f_ins[li]
        for k in range(fin // P):
            nc.sync.dma_start(wl_sb[li][k][:], ins[f"wl{li}"][k * P:(k + 1) * P, :])
            nc.sync.dma_start(wr_sb[li][k][:], ins[f"wr{li}"][k * P:(k + 1) * P, :])
        for j in range(HID // P):
            nc.sync.dma_start(bl_sb[li][:, j:j + 1], ins[f"bl{li}"][j * P:(j + 1) * P, None])

    nc.gpsimd.iota(iota_i[:], pattern=[[1, P]], base=0, channel_multiplier=0)
    nc.vector.tensor_copy(iota_f[:], iota_i[:])
    nc.vector.tensor_copy(iota_b[:], iota_i[:])
    nc.vector.memset(ones_sb[:], 1.0)
    make_identity(nc, ident[:])

    # invrep[p, n] = inv_deg[n] for all p, via rank-1 matmul broadcast
    for (ts, w) in t_chunks:
        invrow_sb = work.tile([1, 512], f, tag="invrow", name="invrow_sb")
        nc.sync.dma_start(invrow_sb[:, :w], ins["invrow"][:, ts:ts + w])
        pb = psA.tile([P, 512], f, tag="hlin", bufs=2, name="pb_inv")
        nc.tensor.matmul(pb[:, :w], lhsT=ones_sb[:, :], rhs=invrow_sb[:, :w],
                         start=True, stop=True)
        nc.scalar.copy(invrep[:, ts:ts + w], pb[:, :w])

    def phase_c(fout_chunks, dt_out, rows, table):
        """transpose hT -> row layout; DMA to rows; AllGather into table.
        If table is None, rows is the final output AP."""
        for b in range(B):
            stage = work.tile([P, fout_chunks * P], dt_out, tag="stage", name="stage")
            for j in range(fout_chunks):
                trp = psA.tile([P, P], f, tag="trp", bufs=4, name="trp_c")
                nc.tensor.transpose(trp[:], hT[j][:, b * P:(b + 1) * P], ident[:])
                nc.vector.tensor_copy(stage[:, j * P:(j + 1) * P], trp[:])
            nc.sync.dma_start(rows[b * P:(b + 1) * P, :], stage[:])
        if table is not None:
            nc.gpsimd.collective_compute(
                "AllGather", mybir.AluOpType.bypass, replica_groups=groups,
                ins=[rows.opt()], outs=[table.opt()])

    def phase_a(fin, table, dt_g, dt_m, iota_t, drel_t):
        nk = fin // P
        wA = C_A * P // 16
        wB = C_B * P // 16
        for b in range(B):
            G = work.tile([P, CT, fin], dt_g, tag="G", name="G")
            # <=512 indices per dma_gather call (SWDGE ring capacity)
            for g in range(C_A // 4):
                nc.gpsimd.dma_gather(
                    G[:, g * 4:(g + 1) * 4, :], table[:, :],
                    idxA_sb[:, b * wA + g * 32: b * wA + (g + 1) * 32],
                    4 * P, 4 * P, fin)
            for g in range(C_B // 4):
                nc.gpsimd.dma_gather(
                    G[:, C_A + g * 4:C_A + (g + 1) * 4, :], table[HALF:, :],
                    idxB_sb[:, b * wB + g * 32: b * wB + (g + 1) * 32],
                    4 * P, 4 * P, fin)
            M = work.tile([P, CT, P], dt_m, tag="M", name="M", bufs=1)
            nc.vector.tensor_tensor(
                out=M[:],
                in0=iota_t[:, None, :].broadcast_to([P, CT, P]),
                in1=drel_t[:, b * CT:(b + 1) * CT][:, :, None].broadcast_to([P, CT, P]),
                op=mybir.AluOpType.is_equal)
            aggp = psA.tile([P, HID], f, tag="agg", bufs=2, name="aggp")
            for c in range(CT):
                nc.tensor.matmul(aggp[:, :fin], lhsT=M[:, c, :], rhs=G[:, c, :],
                                 start=(c == 0), stop=(c == CT - 1))
            aggs = work.tile([P, fin], f, tag="aggs", name="aggs")
            nc.scalar.copy(aggs[:], aggp[:, :fin])
            for k in range(nk):
                trp = psA.tile([P, P], f, tag="trp", bufs=4, name="trp_a")
                nc.tensor.transpose(trp[:], aggs[:, k * P:(k + 1) * P], ident[:])
                nc.vector.tensor_mul(aggT[k][:, b * P:(b + 1) * P], trp[:],
                                     invrep[:, b * P:(b + 1) * P])

    def phase_b(li):
        fin = f_ins[li]
        nk = fin // P
        for (ts, w) in t_chunks:
            ph = [psA.tile([P, 512], f, tag="hlin", bufs=2, name=f"ph{li}_{j}")
                  for j in range(HID // P)]
            for j in range(HID // P):
                n_mm = 2 * nk
                i_mm = 0
                for k in range(nk):
                    nc.tensor.matmul(ph[j][:, :w], lhsT=wl_sb[li][k][:, j * P:(j + 1) * P],
                                     rhs=aggT[k][:, ts:ts + w],
                                     start=(i_mm == 0), stop=(i_mm == n_mm - 1))
                    i_mm += 1
                for k in range(nk):
                    nc.tensor.matmul(ph[j][:, :w], lhsT=wr_sb[li][k][:, j * P:(j + 1) * P],
                                     rhs=hT[k][:, ts:ts + w],
                                     start=(i_mm == 0), stop=(i_mm == n_mm - 1))
                    i_mm += 1
            for j in range(HID // P):
                nc.scalar.activation(hT[j][:, ts:ts + w], ph[j][:, :w],
                                     mybir.ActivationFunctionType.Relu,
                                     bias=bl_sb[li][:, j:j + 1])

    def phase_b0():
        for (ts, w) in t_chunks:
            xT_sb = work.tile([IN_DIM, 512], f, tag="xT", name="xT_sb")
            nc.sync.dma_start(xT_sb[:, :w], ins["xT"][:, ts:ts + w])
            ph = psA.tile([P, 512], f, tag="hlin", bufs=2, name="ph0")
            nc.tensor.matmul(ph[:, :w], lhsT=w_in_sb[:], rhs=xT_sb[:, :w],
                             start=True, stop=True)
            nc.scalar.activation(hT[0][:, ts:ts + w], ph[:, :w],
                                 mybir.ActivationFunctionType.Relu, bias=b_in_sb[:, 0:1])

    def pipeline():
        phase_b0()
        phase_c(EMB // P, f, rows0, table0)
        phase_a(EMB, table0, f, f, iota_f, drel_f)
        phase_b(1)
        phase_c(HID // P, tbl_dt, rows1, table1)
        phase_a(HID, table1, tbl_dt, tbl_dt if lowp else f,
                iota_b if lowp else iota_f, drel_b if lowp else drel_f)
        phase_b(2)
        phase_c(HID // P, tbl_dt, rows2, table2)
        phase_a(HID, table2, tbl_dt, tbl_dt if lowp else f,
                iota_b if lowp else iota_f, drel_b if lowp else drel_f)
        phase_b(3)
        phase_c(HID // P, f, outs["h_out"], None)

    if repeat == 1:
        pipeline()
    else:
        with tc.For_i(0, repeat, 1):
            pipeline()
    ctx.close()


def build_program(cfg: Cfg, C, repeat=1):
    C_A, C_B = C
    CT = C_A + C_B
    nc = bacc.Bacc("TRN2", target_bir_lowering=False, debug=False,
                   enable_asserts=True, num_devices=cfg.R)
    I16 = mybir.dt.int16
    ins = {
        "xT": nc.dram_tensor("xT", [cfg.IN_DIM, cfg.NPB], F32, kind="ExternalInput").ap(),
        "invrow": nc.dram_tensor("invrow", [1, cfg.NPB], F32, kind="ExternalInput").ap(),
        "idxA": nc.dram_tensor("idxA", [P, cfg.B * (C_A * P // 16)], I16, kind="ExternalInput").ap(),
        **({"idxB": nc.dram_tensor("idxB", [P, cfg.B * (C_B * P // 16)], I16,
                                   kind="ExternalInput").ap()} if C_B > 0 else {}),
        "drel": nc.dram_tensor("drel", [P, cfg.B * CT], F32, kind="ExternalInput").ap(),
        "w_in": nc.dram_tensor("w_in", [cfg.IN_DIM, cfg.EMB], F32, kind="ExternalInput").ap(),
        "b_in": nc.dram_tensor("b_in", [cfg.EMB], F32, kind="ExternalInput").ap(),
    }
    for li in range(1, 4):
        fin = cfg.EMB if li == 1 else cfg.HID
        ins[f"wl{li}"] = nc.dram_tensor(f"wl{li}", [fin, cfg.HID], F32, kind="ExternalInput").ap()
        ins[f"bl{li}"] = nc.dram_tensor(f"bl{li}", [cfg.HID], F32, kind="ExternalInput").ap()
        ins[f"wr{li}"] = nc.dram_tensor(f"wr{li}", [fin, cfg.HID], F32, kind="ExternalInput").ap()
    outs = {
        "h_out": nc.dram_tensor("h_out", [cfg.NPB, cfg.HID], F32, kind="ExternalOutput").ap(),
    }
    with tile.TileContext(nc) as tc:
        emit(tc, outs, ins, cfg, C, repeat=repeat)
    nc.compile()
    return nc


def make_runner(nc, in_maps, n_cores):
    """Build a pinned-input PJRT runner: inputs are device_put once, outputs
    are donated back as the next call's (ignored) output buffers, so warm
    calls measure execution + dispatch only."""
    import jax
    from jax.sharding import Mesh, PartitionSpec, NamedSharding
    from jax.experimental.shard_map import shard_map
    from concourse import bass2jax, mybir as mb
    bass2jax.install_neuronx_cc_hook()

    partition_name = nc.partition_id_tensor.name if nc.partition_id_tensor else None
    in_names, out_names, out_avals, zero_outs = [], [], [], []
    for alloc in nc.m.functions[0].allocations:
        if not isinstance(alloc, mb.MemoryLocationSet):
            continue
        name = alloc.memorylocations[0].name
        if alloc.kind == "ExternalInput":
            if name != partition_name:
                in_names.append(name)
        elif alloc.kind == "ExternalOutput":
            shape = tuple(alloc.tensor_shape)
            dtype = mb.dt.np(alloc.dtype)
            out_names.append(name)
            out_avals.append(jax.core.ShapedArray(shape, dtype))
            zero_outs.append(np.zeros(shape, dtype))
    n_params = len(in_names)
    n_outs = len(out_avals)
    all_in_names = list(in_names) + list(out_names)
    if partition_name is not None:
        all_in_names.append(partition_name)
    donate = tuple(range(n_params, n_params + n_outs))

    def _body(*args):
        operands = list(args)
        if partition_name is not None:
            operands.append(bass2jax.partition_id_tensor())
        outs = bass2jax._bass_exec_p.bind(
            *operands,
            out_avals=tuple(out_avals),
            in_names=tuple(all_in_names),
            out_names=tuple(out_names),
            lowering_input_output_aliases=(),
            sim_require_finite=True,
            sim_require_nnan=True,
            nc=nc,
        )
        return tuple(outs)

    devices = jax.devices()[:n_cores]
    mesh = Mesh(np.asarray(devices), ("core",))
    sharded = jax.jit(
        shard_map(_body, mesh=mesh,
                  in_specs=(PartitionSpec("core"),) * (n_params + n_outs),
                  out_specs=(PartitionSpec("core"),) * n_outs,
                  check_rep=False),
        donate_argnums=donate, keep_unused=True)
    sh = NamedSharding(mesh, PartitionSpec("core"))
    concat_in = [np.concatenate([np.asarray(m[nm]) for m in in_maps], axis=0)
                 for nm in in_names]
    d_in = [jax.device_put(a, sh) for a in concat_in]
    state = {"donate": [jax.device_put(
        np.zeros((n_cores * z.shape[0], *z.shape[1:]), z.dtype), sh) for z in zero_outs]}

    def run():
        outs = sharded(*d_in, *state["donate"])
        jax.block_until_ready(outs)
        state["donate"] = list(outs)
        return outs

    def results():
        outs = state["donate"]
        return [{nm: np.asarray(outs[i]).reshape(n_cores, *out_avals[i].shape)[c]
                 for i, nm in enumerate(out_names)} for c in range(n_cores)]

    return run, results


def kernel(**inputs) -> np.ndarray:
    cfg = Cfg()
    layers = [(inputs["Wl1"], inputs["bl1"], inputs["Wr1"]),
              (inputs["Wl2"], inputs["bl2"], inputs["Wr2"]),
              (inputs["Wl3"], inputs["bl3"], inputs["Wr3"])]
    in_maps, C = host_prep(cfg, inputs["x"], inputs["edge_index"],
                           inputs["W_in"], inputs["b_in"], layers)
    nc = build_program(cfg, C)
    # the NTFF trace hook (antenv.axon_hooks) is absent in this container;
    # make sure run_bass_kernel_spmd never takes the trace path.
    os.environ["BASS_NEVER_TRACE"] = "1"
    res = bass_utils.run_bass_kernel_spmd(
        nc, in_maps, core_ids=list(range(cfg.R)), trace=False)
    if res.exec_time_ns is not None:
        print(f"HW exec time: {res.exec_time_ns} ns")
    out = np.concatenate(
        [res.results[r]["h_out"][:cfg.NPC] for r in range(cfg.R)], axis=0)
    return out.astype(np.float32)


if __name__ == "__main__":
    # smoke: build only
    cfg = Cfg()
    rng = np.random.default_rng(0)
    ei = rng.integers(0, cfg.N, size=(2, cfg.E), dtype=np.int64)
    x = rng.standard_normal((cfg.N, cfg.IN_DIM), dtype=np.float32)
    layers = [(rng.standard_normal((cfg.EMB, cfg.HID), dtype=np.float32) * 0.1,
               np.zeros(cfg.HID, np.float32),
               rng.standard_normal((cfg.EMB, cfg.HID), dtype=np.float32) * 0.1)]
    layers += [(rng.standard_normal((cfg.HID, cfg.HID), dtype=np.float32) * 0.1,
                np.zeros(cfg.HID, np.float32),
                rng.standard_normal((cfg.HID, cfg.HID), dtype=np.float32) * 0.1) for _ in range(2)]
    in_maps, C = host_prep(cfg, x, ei, rng.standard_normal((cfg.IN_DIM, cfg.EMB), dtype=np.float32),
                           np.zeros(cfg.EMB, np.float32), layers)
    print("C =", C)
    nc = build_program(cfg, C)
    print("built ok; instructions:",
          sum(len(bb.instructions) for fn in nc.m.functions for bb in fn.blocks))

